# revision 1
# baseline (speedup 1.0000x reference)
"""Trainium2 Bass kernel for nn_Attention_730144440595 (NormAttention block).

8 NeuronCores, data-parallel over batch (16 -> 2/core). Per core:
  - channel-LN folded: x centered on-device (mean via PE ones-matmuls, broadcast
    via K=1 matmul); per-pixel rstd folded into the softmax exp as a log-bias.
  - QKV GEMM with host-padded w_qkv^T so each k-head lands in its own
    zero-padded 128-row tile (K=128 full-array sims, no row-group games).
  - q L2-norm applied via PE-broadcast multiply; k-norm and the x8 scale fold
    into the exp's per-partition scale.
  - attention transposed (sim^T[j,i]): exp once per element; attn@V contracts
    over partitions with a std-column appended to V so the softmax denominator
    drops out of the same matmul.  bf16 matmuls, f32 PSUM.
"""

import sys
import types

import numpy as np

B = 2
C = 256
N = 1024
HEADS = 4
D = 64
P = 128
NCORES = 8
LN_EPS = 1e-5
LOG8 = float(np.log(8.0))
NOT = 8          # qkv o-tiles: q 0-1, k(padded) 2-5, v 6-7
OW = NOT * P     # padded qkv rows = 1024


def _host_consts():
    cst = np.zeros((P, 16), np.float32)
    # E_ind[cc][p, h] = 1 iff h == 2*cc + p//64   (cols 0-3 / 4-7)
    for cc in range(2):
        for p in range(P):
            cst[p, 4 * cc + 2 * cc + p // 64] = 1.0
    cst[:, 8] = 1.0 / 256.0    # rhs_x col0 (mean)
    cst[:, 11] = 1.0 / 256.0   # rhs_q col1 (msq)  (cols 10-11)
    cst[:, 12] = -1.0 / 256.0  # negones
    cst[:64, 13] = 1.0         # khalf indicator (rows 0-63)
    cst[:, 14] = 1.0           # ones col
    cst[64:, 15] = 1.0         # khalf_odd (rows 64-127)
    cst4 = np.zeros((HEADS, 2 * P), np.float32)
    for cc in range(2):
        for m in range(P):
            cst4[2 * cc + m // 64, cc * P + m] = 1.0
    return cst, cst4


def _host_weights(w_qkv, w_out):
    """wt_pad [C, 1024]: q cols 0-255, k head h at cols 256+128h..+64, v 768-1023.
    wot_pad [512, C]: head h rows 128h..128h+64 = w_out^T[h*64:(h+1)*64]."""
    import ml_dtypes
    wt = np.ascontiguousarray(np.asarray(w_qkv, np.float32).T)  # [C, 768]
    wt_pad = np.zeros((C, OW), np.float32)
    wt_pad[:, 0:256] = wt[:, 0:256]
    for h in range(HEADS):
        off = 256 + 128 * h + (h % 2) * D
        wt_pad[:, off:off + D] = wt[:, 256 + D * h:256 + D * (h + 1)]
    wt_pad[:, 768:1024] = wt[:, 512:768]
    wot = np.ascontiguousarray(np.asarray(w_out, np.float32).T)  # [ci, co]
    wot_pad = np.zeros((4 * P, C), np.float32)
    for h in range(HEADS):
        wot_pad[128 * h:128 * h + D, :] = wot[D * h:D * (h + 1), :]
    return wt_pad.astype(ml_dtypes.bfloat16), wot_pad.astype(ml_dtypes.bfloat16)


def _install_ntff_hook():
    try:
        import antenv
        if getattr(antenv, "axon_hooks", None) is not None:
            return
        from trn_agent_boot.trn_boot import _ntff_profile_via_ctypes
        hook = _ntff_profile_via_ctypes('/opt/axon/libaxon_pjrt.so')
        mod = types.ModuleType('antenv.axon_hooks')
        mod._hook = hook
        mod.get_axon_ntff_profile_hook = lambda: mod._hook
        mod.set_axon_ntff_profile_hook = lambda h: setattr(mod, '_hook', h)
        sys.modules['antenv.axon_hooks'] = mod
        antenv.axon_hooks = mod
    except Exception:
        pass


def build_nc():
    import concourse.bass as bass
    import concourse.tile as tile
    import concourse.mybir as mybir
    from concourse import bacc
    from contextlib import ExitStack

    dt = mybir.dt
    f32 = dt.float32
    bf16 = dt.bfloat16
    AF = mybir.ActivationFunctionType
    OP = mybir.AluOpType

    # Steer the ACT table selector: keep Exp/Ln only in the combined set so
    # the kernel never thrashes between exp_and_others and natural_log sets
    # (each switch costs ~2.6us of ACT_TABLE_LOAD + drain).
    from concourse.hw_specs import get_activation_tables
    _tabs = get_activation_tables("gen3")
    for _name, _fns in _tabs.items():
        if _name != "natural_log_exp_and_others":
            _fns.discard(AF.Exp)
            _fns.discard(AF.Ln)

    nc = bacc.Bacc("TRN2", target_bir_lowering=False, num_devices=NCORES)
    x_d = nc.dram_tensor("x", [B, C, N], f32, kind="ExternalInput").ap()
    wt_d = nc.dram_tensor("wt", [C, OW], bf16, kind="ExternalInput").ap()
    wot_d = nc.dram_tensor("wot", [4 * P, C], bf16, kind="ExternalInput").ap()
    xbf_d = nc.dram_tensor("xbf", [B, C, N], bf16, kind="ExternalInput").ap()
    g_d = nc.dram_tensor("g", [C], f32, kind="ExternalInput").ap()
    cst_d = nc.dram_tensor("cst", [P, 16], f32, kind="ExternalInput").ap()
    cst4_d = nc.dram_tensor("cst4", [HEADS, 2 * P], f32, kind="ExternalInput").ap()
    out_d = nc.dram_tensor("out", [B, C, N], f32, kind="ExternalOutput").ap()

    with tile.TileContext(nc) as tc, ExitStack() as ctx:
        const = ctx.enter_context(tc.tile_pool(name="const", bufs=1))
        big = ctx.enter_context(tc.tile_pool(name="big", bufs=1))
        tmp = ctx.enter_context(tc.tile_pool(name="tmp", bufs=2))
        expp = ctx.enter_context(tc.tile_pool(name="expp", bufs=6))
        psA = ctx.enter_context(tc.tile_pool(name="psA", bufs=3, space="PSUM"))
        psB = ctx.enter_context(tc.tile_pool(name="psB", bufs=2, space="PSUM"))

        def mm(out, lhsT, rhs, start, stop):
            nc.tensor.matmul(out, lhsT, rhs, start=start, stop=stop)

        # ---------------- constants ----------------
        cst_f = tmp.tile([P, 16], f32, tag="cst_f", name="cst_f")
        nc.sync.dma_start(cst_f, cst_d[:])
        cst = const.tile([P, 16], bf16, tag="cst", name="cst")
        nc.vector.tensor_copy(out=cst[:], in_=cst_f[:])
        E_ind = [cst[:, 0:4], cst[:, 4:8]]
        rhs_x = cst[:, 8:10]
        rhs_q = cst[:, 10:12]
        negones = cst[:, 12:13]
        khalf = [cst[:, 13:14], cst[:, 15:16]]
        cst4_f = tmp.tile([HEADS, 2 * P], f32, tag="cst4_f", name="cst4_f")
        nc.sync.dma_start(cst4_f, cst4_d[:])
        cst4 = const.tile([HEADS, 2 * P], bf16, tag="cst4", name="cst4")
        nc.vector.tensor_copy(out=cst4[:], in_=cst4_f[:])
        E4 = [cst4[:, 0:128], cst4[:, 128:256]]

        ones2 = const.tile([P, D], bf16, tag="ones2", name="ones2")
        nc.vector.memset(ones2[:], 1.0)
        ones_row = const.tile([1, P], bf16, tag="ones_row", name="ones_row")
        nc.vector.memset(ones_row[:], 1.0)
        eps_col = const.tile([P, 1], f32, tag="eps_col", name="eps_col")
        nc.vector.memset(eps_col[:], LN_EPS)
        log8_col = const.tile([P, 1], f32, tag="log8_col", name="log8_col")
        nc.vector.memset(log8_col[:], LOG8)


        # ---------------- loads + weight prep ----------------
        x_sb = [[big.tile([P, N], f32, tag=f"x{b}{cc}", name=f"x{b}{cc}")
                 for cc in range(2)] for b in range(B)]

        wg_sb = big.tile([P, 2, OW], bf16, tag="wg", name="wg")
        nc.sync.dma_start(wg_sb, wt_d.rearrange("(cc p) o -> p cc o", p=P))
        g_sb = const.tile([P, 2], f32, tag="g", name="g")
        nc.sync.dma_start(g_sb, g_d.rearrange("(cc p) -> p cc", p=P))
        for cc in range(2):
            nc.vector.tensor_scalar_mul(wg_sb[:, cc], wg_sb[:, cc], g_sb[:, cc:cc + 1])

        wot_h = []
        for h in range(HEADS):
            wb = big.tile([P, C], bf16, tag=f"wot{h}", name=f"wot{h}")
            nc.sync.dma_start(wb, wot_d[h * P:(h + 1) * P, :])
            wot_h.append(wb)

        x_bf = [[big.tile([P, N], bf16, tag=f"xbf{b}{cc}", name=f"xbf{b}{cc}")
                 for cc in range(2)] for b in range(B)]
        xsq = [[big.tile([P, N], bf16, tag=f"xsq{b}{cc}", name=f"xsq{b}{cc}")
                for cc in range(2)] for b in range(B)]
        for b in range(B):
            for cc in range(2):
                nc.sync.dma_start(x_bf[b][cc], xbf_d[b, cc * P:(cc + 1) * P, :])
                nc.vector.tensor_mul(xsq[b][cc][:], x_bf[b][cc][:], x_bf[b][cc][:])

        # ---------------- LN stats + centering (per batch) ----------------
        logrstd = [big.tile([P, 8], f32, tag=f"lrs{b}", name=f"lrs{b}") for b in range(B)]
        std_sb = [big.tile([P, 8], f32, tag=f"std{b}", name=f"std{b}") for b in range(B)]
        negmean_row = [big.tile([1, N], bf16, tag=f"nmr{b}", name=f"nmr{b}") for b in range(B)]

        for b in range(B):
            st_ps = psA.tile([P, 8, 2], f32, tag="A", name="st_ps")
            for ic in range(8):
                sl = st_ps[:, ic]
                mm(sl, x_bf[b][0][:, ic * P:(ic + 1) * P], rhs_x, True, False)
                mm(sl, x_bf[b][1][:, ic * P:(ic + 1) * P], rhs_x, False, False)
                mm(sl, xsq[b][0][:, ic * P:(ic + 1) * P], rhs_q, False, False)
                mm(sl, xsq[b][1][:, ic * P:(ic + 1) * P], rhs_q, False, True)
            st_sb = tmp.tile([P, 8, 2], f32, tag="st_sb", name="st_sb")
            nc.vector.tensor_copy(out=st_sb[:], in_=st_ps[:])
            mean_v = st_sb[:, :, 0]
            msq_v = st_sb[:, :, 1]
            m2 = tmp.tile([P, 8], f32, tag="m2", name="m2")
            nc.vector.tensor_mul(m2[:], mean_v, mean_v)
            var = tmp.tile([P, 8], f32, tag="var", name="var")
            nc.vector.tensor_sub(var[:], msq_v, m2[:])
            lnv = tmp.tile([P, 8], f32, tag="lnv", name="lnv")
            nc.scalar.activation(lnv[:], var[:], AF.Ln, bias=eps_col[:])
            nc.vector.tensor_scalar_mul(logrstd[b][:], lnv[:], -0.5)
            rstd = tmp.tile([P, 8], f32, tag="rstd", name="rstd")
            nc.scalar.activation(rstd[:], logrstd[b][:], AF.Exp)
            nc.vector.scalar_tensor_tensor(
                std_sb[b][:], var[:], LN_EPS, rstd[:], op0=OP.add, op1=OP.mult)
            # negmean row + broadcast + center x in-place
            for ih in range(2):
                io = ih * 512
                nm_ps = psA.tile([1, 512], f32, tag="A", name="nm_ps")
                for cc in range(2):
                    mm(nm_ps[:], negones, x_bf[b][cc][:, io:io + 512],
                       start=(cc == 0), stop=(cc == 1))
                nc.vector.tensor_copy(out=negmean_row[b][:, io:io + 512], in_=nm_ps[:])
                nmbc_ps = psA.tile([P, 512], f32, tag="A", name="nmbc_ps")
                mm(nmbc_ps[:], ones_row[:],
                   negmean_row[b][:, io:io + 512], True, True)
                for cc in range(2):
                    nc.vector.tensor_add(x_bf[b][cc][:, io:io + 512],
                                         x_bf[b][cc][:, io:io + 512], nmbc_ps[:])

        # ---------------- QKV GEMM ----------------
        # o-tiles: 0-1 q, 2-5 k per head (rows 64-127 zero), 6-7 v
        qkv_sb = [[big.tile([P, N], bf16, tag=f"qkv{b}{ot}", name=f"qkv{b}{ot}")
                   for ot in range(NOT)] for b in range(B)]
        def _qkv_phase(b, ots=(6, 7, 0, 1, 2, 3, 4, 5)):
            for ot in ots:
                qk_ps = psA.tile([P, N], f32, tag="A", name="qk_ps")
                for ih in range(2):
                    io = ih * 512
                    mm(qk_ps[:, io:io + 512], wg_sb[:, 0, ot * P:(ot + 1) * P],
                       x_bf[b][0][:, io:io + 512], True, False)
                    mm(qk_ps[:, io:io + 512], wg_sb[:, 1, ot * P:(ot + 1) * P],
                       x_bf[b][1][:, io:io + 512], False, True)
                if ot < 2 or b == 1:
                    nc.vector.tensor_copy(out=qkv_sb[b][ot][:], in_=qk_ps[:])
                else:
                    nc.scalar.copy(out=qkv_sb[b][ot][:], in_=qk_ps[:])

        # ---------------- q/k norms ----------------
        b8_sb = [big.tile([P, 8, HEADS], f32, tag=f"b8{b}", name=f"b8{b}") for b in range(B)]
        a_sb = [tmp.tile([HEADS, N], bf16, tag="a_sb", name=f"a_sb{b}") for b in range(B)]
        def _norm_phase(b):
            qsq = [tmp.tile([P, N], bf16, tag=f"qsq{cc}", name=f"qsq{cc}") for cc in range(2)]
            for cc in range(2):
                nc.vector.tensor_mul(qsq[cc][:], qkv_sb[b][cc][:], qkv_sb[b][cc][:])
            ksq = [tmp.tile([P, N], bf16, tag=f"ksq{h}", name=f"ksq{h}") for h in range(HEADS)]
            for h in range(HEADS):
                nc.vector.tensor_mul(ksq[h][:], qkv_sb[b][2 + h][:],
                                     qkv_sb[b][2 + h][:])
            a_ln = tmp.tile([HEADS, N], f32, tag="a_ln", name="a_ln")
            for ih in range(2):
                io = ih * 512
                s2q_ps = psA.tile([HEADS, 512], f32, tag="A", name="s2q_ps")
                for cc in range(2):
                    mm(s2q_ps[:], E_ind[cc], qsq[cc][:, io:io + 512],
                       start=(cc == 0), stop=(cc == 1))
                nc.scalar.activation(a_ln[:, io:io + 512], s2q_ps[:], AF.Ln)
            nc.scalar.activation(a_sb[b][:], a_ln[:], AF.Exp, scale=-0.5)
            bsq_ps = psA.tile([P, 8, HEADS], f32, tag="A", name="bsq_ps")
            for jc in range(8):
                for h in range(HEADS):
                    mm(bsq_ps[:, jc, h:h + 1], ksq[h][:, jc * P:(jc + 1) * P],
                       khalf[h % 2], True, True)
            b8ln = tmp.tile([P, 8, HEADS], f32, tag="b8ln", name="b8ln")
            nc.scalar.activation(b8ln[:], bsq_ps[:], AF.Ln)
            nc.scalar.activation(b8_sb[b][:], b8ln[:], AF.Exp, scale=-0.5,
                                 bias=log8_col[:])
            for cc in range(2):
                for ih in range(2):
                    io = ih * 512
                    abc_ps = psA.tile([P, 512], f32, tag="A", name="abc_ps")
                    mm(abc_ps[:], E4[cc], a_sb[b][:, io:io + 512], True, True)
                    nc.vector.tensor_mul(qkv_sb[b][cc][:, io:io + 512],
                                         qkv_sb[b][cc][:, io:io + 512], abc_ps[:])

        # ---------------- V transpose (DMA xbar) + augment ----------------
        vaug = [[[big.tile([P, 2, D + 1], bf16, tag=f"va{b}{cc}{jc}",
                           name=f"va{b}{cc}{jc}")
                  for jc in range(8)] for cc in range(2)] for b in range(B)]
        def _vaug_phase(b):
            for cc in range(2):
                for jc in range(8):
                    vt = tmp.tile([P, P], bf16, tag="vt", name="vt")
                    nc.sync.dma_start_transpose(
                        vt[:], qkv_sb[b][6 + cc][:, jc * P:(jc + 1) * P])
                    va = vaug[b][cc][jc]
                    nc.gpsimd.tensor_copy(
                        out=va[:, :, 0:D],
                        in_=vt.rearrange("p (hh d) -> p hh d", hh=2))
                    nc.gpsimd.tensor_copy(
                        out=va[:, :, D:D + 1],
                        in_=std_sb[b][:, jc:jc + 1, None].to_broadcast((P, 2, 1)))

        _qkv_phase(0)
        _vaug_phase(0)
        _norm_phase(0)

        # ---------------- attention + per-batch epilogue ----------------
        u65 = [[big.tile([P, N], bf16, tag=f"u{b}{h}", name=f"u{b}{h}")
                for h in range(HEADS)] for b in range(B)]
        for b in range(B):
            for h in range(HEADS):
                # zero the padding rows once (row 64 overwritten by S later)
                nc.gpsimd.memset(u65[b][h][D:P, :], 0.0)
        for b in range(B):
            for cc in range(2):
                nc.sync.dma_start(x_sb[b][cc], x_d[b, cc * P:(cc + 1) * P, :])
        def _att_head(b, h):
                if b == 0:
                    if h == 1:
                        _qkv_phase(1, (6, 7, 0, 1))
                    elif h == 2:
                        _qkv_phase(1, (2, 3, 4, 5))
                        _vaug_phase(1)
                    elif h == 3:
                        _norm_phase(1)
                cc, hh = h // 2, h % 2
                U_ps = [psB.tile([D + 1, 512], f32, tag="B", name=f"U{ih}")
                        for ih in range(2)]
                for jc in range(8):
                    sim_ps = psA.tile([P, N], f32, tag="A", name="sim")
                    for ih in range(2):
                        mm(sim_ps[:, ih * 512:(ih + 1) * 512],
                           qkv_sb[b][2 + h][:, jc * P:(jc + 1) * P],
                           qkv_sb[b][cc][:, ih * 512:(ih + 1) * 512],
                           True, True)
                    et = expp.tile([P, N], bf16, tag="et", name="et")
                    nc.scalar.activation(et[:], sim_ps[:], AF.Exp,
                                         bias=logrstd[b][:, jc:jc + 1],
                                         scale=b8_sb[b][:, jc, h:h + 1])
                    for ih in range(2):
                        mm(U_ps[ih][:], vaug[b][cc][jc][:, hh],
                           et[:, ih * 512:(ih + 1) * 512],
                           start=(jc == 0), stop=(jc == 7))
                for ih in range(2):
                    nc.vector.tensor_copy(
                        out=u65[b][h][0:D + 1, ih * 512:(ih + 1) * 512],
                        in_=U_ps[ih][:])

        def _epi_h(b, h):
            for ih in range(2):
                io = ih * 512
                sbc_ps = psA.tile([D, 512], f32, tag="A", name="sbc_ps")
                mm(sbc_ps[:], ones2[D:D + 1, :],
                   u65[b][h][D:D + 1, io:io + 512], True, True)
                rbc = tmp.tile([D, 512], f32, tag="rbc", name="rbc")
                nc.vector.reciprocal_approx_fast(out=rbc[:], in_=sbc_ps[:])
                nc.gpsimd.tensor_mul(u65[b][h][0:D, io:io + 512],
                                     u65[b][h][0:D, io:io + 512], rbc[:])

        def _proj(b):
            for co in range(2):
                out_f = tmp.tile([P, N], f32, tag="out_f", name="out_f")
                for ih in range(2):
                    io = ih * 512
                    out_ps = psA.tile([P, 512], f32, tag="A", name="out_ps")
                    for h in range(HEADS):
                        mm(out_ps[:], wot_h[h][:, co * P:(co + 1) * P],
                           u65[b][h][:, io:io + 512],
                           start=(h == 0), stop=(h == 3))
                    nc.vector.tensor_add(out_f[:, io:io + 512],
                                         out_ps[:], x_sb[b][co][:, io:io + 512])
                nc.sync.dma_start(out_d[b, co * P:(co + 1) * P, :], out_f[:])

        for h in range(HEADS):
            _att_head(0, h)
        _att_head(1, 0)
        _epi_h(0, 0)
        _att_head(1, 1)
        _epi_h(0, 1)
        _epi_h(1, 0)
        _att_head(1, 2)
        _epi_h(0, 2)
        _epi_h(1, 1)
        _att_head(1, 3)
        _epi_h(0, 3)
        _proj(0)
        _epi_h(1, 2)
        _epi_h(1, 3)
        _proj(1)

    nc.compile()
    return nc


_NC = None
last_exec_time_ns = None


def _get_nc():
    global _NC
    if _NC is None:
        _NC = build_nc()
    return _NC


def _run(in_maps, trace=False):
    global last_exec_time_ns
    from concourse.bass_utils import run_bass_kernel_spmd
    nc = _get_nc()
    if trace:
        _install_ntff_hook()
    try:
        res = run_bass_kernel_spmd(nc, in_maps, core_ids=list(range(NCORES)),
                                   trace=trace)
    except Exception:
        if not trace:
            raise
        res = run_bass_kernel_spmd(nc, in_maps, core_ids=list(range(NCORES)),
                                   trace=False)
    last_exec_time_ns = res.exec_time_ns
    return res


def kernel(x, g, w_qkv, w_out, _trace=False):
    x = np.ascontiguousarray(np.asarray(x, dtype=np.float32))
    g = np.asarray(g, dtype=np.float32).reshape(C)
    wt_pad, wot_pad = _host_weights(w_qkv, w_out)
    b_full, c, H, W = x.shape
    assert (b_full, c, H * W) == (NCORES * B, C, N)
    xr = x.reshape(b_full, C, N)
    cst, cst4 = _host_consts()
    in_maps = []
    for i in range(NCORES):
        import ml_dtypes as _md
        in_maps.append({
            "x": np.ascontiguousarray(xr[i * B:(i + 1) * B]),
            "xbf": np.ascontiguousarray(xr[i * B:(i + 1) * B]).astype(_md.bfloat16),
            "wt": wt_pad,
            "wot": wot_pad,
            "g": g,
            "cst": cst,
            "cst4": cst4,
        })
    res = _run(in_maps, trace=_trace)
    out = np.concatenate([res.results[i]["out"] for i in range(NCORES)], axis=0)
    return out.reshape(b_full, C, H, W).astype(np.float32)

